# revision 7
# baseline (speedup 1.0000x reference)
"""GNN message-passing (std aggregator) on 8 TRN2 NeuronCores.

Math per target node: count, S1 = sum x[src], S2 = sum x[src]^2;
mean = S1/max(count,eps); var = S2/count - mean^2;
std = sqrt(max(var,0)), zeroed where count <= 1.

Strategy: shard TARGET nodes across cores (no collectives). Host packs nodes
into 128-bin blocks balanced by in-degree (serpentine deal), buckets edges by
(block, src-half) with uniform tile capacity th per (block,half) so one NEFF
serves all cores. Source rows are addressed with the int16 OFFSET trick:
gather base = table + (32768 + h*50000) rows, idx in [-32768, 17231], so each
half covers 50000 rows (vs 25000 with unsigned int16), cutting slot padding
from 25% to 13%. Host pre-packs a [N,128] bf16 table xpack = [x | x^2]
(256B gather rows) and per-node rz = (count>1)/max(count,eps).
Device per core, per group of GB blocks:
  - 2x dma_gather on alternating swdge queue pairs ({0,1} / {2,3} by group
    parity) so all 4 Q7 core pairs generate descriptors concurrently,
  - DVE builds 4-wide one-hot tiles (slot-vs-iota is_equal),
  - PE matmul-accumulates [128 bins x 128] = [S1 | S2] in PSUM,
  - ACT applies rz while copying PSUM->SBUF, DVE batched var, ACT sqrt,
    one DMA out per group.
"""

import numpy as np
import ml_dtypes

N_NODES = 100000
N_FEAT = 64
N_EDGES = 1600000
P = 128
NCORES = 8
NB = 98                 # blocks per core
NBLK = NCORES * NB      # 784
GB = 7                  # blocks per group; 98 = 14*7
NHALF = 2
NH = N_NODES // NHALF   # 50000 rows per half (int16 offset trick)
IOFF = 32768            # gather base offset rows
TROWS = IOFF + NH + 32768  # padded table rows: max addr = 32768+50000+32767
EPS = 1e-8
BF16 = ml_dtypes.bfloat16

_CACHE = {}


def _build_program(f, nb, th, gb, nh):
    import concourse.bass as bass
    import concourse.bacc as bacc
    import concourse.mybir as mybir
    import concourse.tile as tile

    F32 = mybir.dt.float32
    I16 = mybir.dt.int16
    BF = mybir.dt.bfloat16
    AO = mybir.AluOpType

    w = 2 * f                  # 128 = [x | x^2]
    t = NHALF * th             # tiles (columns of 128 edges) per block
    C = nb * t                 # total 128-edge packets per core
    gcols = gb * t             # packets per group
    qcols = gb * th            # packets per (group, half)
    ng = nb // gb
    nidx = qcols * P           # indices per gather
    i16c = nidx // 16          # idx16 cols per gather

    nc = bacc.Bacc(num_swdge_queues=4)
    xd = nc.declare_dram_parameter("xpack", [TROWS, w], BF, isOutput=False)
    gidxd = nc.declare_dram_parameter(
        "gidx", [P, ng * NHALF * i16c], I16, isOutput=False)
    tgtd = nc.declare_dram_parameter("tgt", [P, C], F32, isOutput=False)
    rzd = nc.declare_dram_parameter("rz", [P, nb], F32, isOutput=False)
    outd = nc.declare_dram_parameter("out", [ng * P, gb * f], F32,
                                     isOutput=True)

    with tile.TileContext(nc) as tc:
        with (
            tc.tile_pool(name="const", bufs=1) as constp,
            tc.tile_pool(name="io", bufs=2) as iop,
            tc.tile_pool(name="msg", bufs=2) as msgp,
            tc.tile_pool(name="oh", bufs=12) as ohp,
            tc.tile_pool(name="fin", bufs=2) as finp,
            tc.tile_pool(name="ov", bufs=2) as ovp,
            tc.tile_pool(name="ps", bufs=8, space="PSUM") as psump,
        ):
            # 4-wide iota [128, 4*128]: value = column index % 128
            iota4 = constp.tile([P, 4 * P], F32)
            nc.gpsimd.iota(iota4[:], pattern=[[0, 4], [1, P]], base=0,
                           channel_multiplier=0,
                           allow_small_or_imprecise_dtypes=True)
            rz = constp.tile([P, nb], F32)
            nc.sync.dma_start(out=rz[:], in_=rzd[:, :])

            for g in range(ng):
                idx = iop.tile([P, NHALF * i16c], I16, tag="idx")
                tg = iop.tile([P, gcols], F32, tag="tg")
                nc.sync.dma_start(
                    out=idx[:],
                    in_=gidxd[:, g * NHALF * i16c:(g + 1) * NHALF * i16c])
                nc.sync.dma_start(
                    out=tg[:], in_=tgtd[:, g * gcols:(g + 1) * gcols])

                sqx = msgp.tile([P, gcols * w], BF, tag="sqx")
                s3 = sqx[:].rearrange("p (c e) -> p c e", e=w)
                for h in range(NHALF):
                    base = IOFF + h * nh
                    nc.gpsimd.dma_gather(
                        out_ap=s3[:, h * qcols:(h + 1) * qcols, :],
                        in_ap=xd[base:base + 2, :],
                        idxs_ap=idx[:, h * i16c:(h + 1) * i16c],
                        num_idxs=nidx,
                        num_idxs_reg=nidx,
                        elem_size=w,
                        single_packet=False,
                        queue_num=2 * (g % 2) + h,
                    )

                pss = [psump.tile([P, w], F32, tag="ps", name=f"ps_{g}_{bl}")
                       for bl in range(gb)]
                for pk in range((gcols + 3) // 4):
                    npk = min(4, gcols - 4 * pk)
                    oh4 = ohp.tile([P, 4 * P], BF)
                    nc.vector.tensor_tensor(
                        out=oh4[:, 0:npk * P]
                            .rearrange("p (c e) -> p c e", e=P),
                        in0=tg[:, 4 * pk:4 * pk + npk]
                            .rearrange("p (c u) -> p c u", u=1)
                            .to_broadcast([P, npk, P]),
                        in1=iota4[:, 0:npk * P]
                            .rearrange("p (c e) -> p c e", e=P),
                        op=AO.is_equal,
                    )
                    for i in range(npk):
                        cl = 4 * pk + i
                        h = cl // qcols
                        r = cl % qcols
                        bl = r // th
                        j = r % th
                        nc.tensor.matmul(
                            out=pss[bl][:],
                            lhsT=oh4[:, i * P:(i + 1) * P],
                            rhs=sqx[:, cl * w:(cl + 1) * w],
                            start=(h == 0 and j == 0),
                            stop=(h == NHALF - 1 and j == th - 1),
                        )

                # finishing: ACT copies PSUM->SBUF scaled by rz, then DVE
                # batched var over [P, gb*f], ACT sqrt, one DMA per group
                me = finp.tile([P, gb * w], F32, tag="me")
                m3 = me[:].rearrange("p (b e) -> p b e", e=w)
                for bl in range(gb):
                    b = g * gb + bl
                    nc.scalar.mul(
                        out=me[:, bl * w:(bl + 1) * w], in_=pss[bl][:],
                        mul=rz[:, b:b + 1])
                var = finp.tile([P, gb * f], F32, tag="var")
                v3 = var[:].rearrange("p (b e) -> p b e", e=f)
                nc.vector.tensor_tensor(
                    out=v3[:, :, :], in0=m3[:, :, 0:f], in1=m3[:, :, 0:f],
                    op=AO.mult)
                nc.vector.tensor_tensor(
                    out=v3[:, :, :], in0=m3[:, :, f:w], in1=v3[:, :, :],
                    op=AO.subtract)
                nc.vector.tensor_scalar(
                    out=var[:], in0=var[:], scalar1=0.0, scalar2=None,
                    op0=AO.max)
                std = ovp.tile([P, gb * f], F32, tag="std")
                nc.scalar.sqrt(out=std[:], in_=var[:])
                nc.sync.dma_start(
                    out=outd[g * P:(g + 1) * P, :], in_=std[:])
    return nc


def _host_prep(x, edge_index):
    src = np.asarray(edge_index[0], dtype=np.int64)
    tgt = np.asarray(edge_index[1], dtype=np.int64)
    n_edges = src.shape[0]
    counts = np.bincount(tgt, minlength=N_NODES)

    # serpentine deal of count-sorted nodes into NBLK blocks of <=128 slots
    order = np.argsort(-counts, kind="stable")
    ranks = np.arange(N_NODES)
    rounds = ranks // NBLK
    pos = ranks % NBLK
    blk_of_rank = np.where(rounds % 2 == 0, pos, NBLK - 1 - pos)
    blk = np.empty(N_NODES, np.int64)
    slot = np.empty(N_NODES, np.int64)
    blk[order] = blk_of_rank
    slot[order] = rounds
    assert slot.max() < P

    eb = blk[tgt]                      # edge -> block
    eh = src // NH                     # edge -> src half
    es = slot[tgt]                     # edge -> slot in block
    seg = eb * NHALF + eh              # edge -> (block, half) segment
    segsums = np.bincount(seg, minlength=NBLK * NHALF)
    th = int(np.ceil(segsums.max() / P))
    cap = th * P

    # within each segment, order edges by src row for DRAM gather locality
    order_e = np.lexsort((src, seg))
    segs = seg[order_e]
    starts = np.zeros(NBLK * NHALF, np.int64)
    np.cumsum(segsums[:-1], out=starts[1:])
    within = np.arange(n_edges) - starts[segs]
    flat = segs * cap + within

    # idx values use the int16 offset trick: row r of half -> r - 32768;
    # padding slots use 0 (a valid row; one-hot column is all-zero)
    gidxq = np.zeros((NBLK, NHALF, cap), np.int16)
    tgtq = np.full((NBLK, NHALF, cap), -1.0, np.float32)
    gidxq.reshape(-1)[flat] = (src[order_e] % NH - IOFF).astype(np.int16)
    tgtq.reshape(-1)[flat] = es[order_e].astype(np.float32)

    # trailing-pop guard: the gather ucode drops trailing negative idxs from
    # each stream; ensure the final slot of every (core, group, half) stream
    # (= last block of the group, tile th-1, pos 127) has idx >= 0 by
    # swapping within its segment (edges may occupy any slot of their seg).
    ng = NB // GB
    for c in range(NCORES):
        for g in range(ng):
            b = c * NB + g * GB + GB - 1
            for h in range(NHALF):
                if gidxq[b, h, cap - 1] < 0 and tgtq[b, h, cap - 1] >= 0:
                    cand = np.nonzero(gidxq[b, h] >= 0)[0]
                    assert cand.size > 0, "no swap partner for pop guard"
                    jj = cand[0]
                    gidxq[b, h, cap - 1], gidxq[b, h, jj] = (
                        gidxq[b, h, jj], gidxq[b, h, cap - 1])
                    tgtq[b, h, cap - 1], tgtq[b, h, jj] = (
                        tgtq[b, h, jj], tgtq[b, h, cap - 1])

    # packed per-node table [x | x^2] in bf16 (256B rows), padded for the
    # offset addressing window
    xf = np.asarray(x, dtype=np.float32)
    xpack = np.zeros((TROWS, 2 * N_FEAT), BF16)
    xpack[:N_NODES, :N_FEAT] = xf.astype(BF16)
    xpack[:N_NODES, N_FEAT:] = (xf * xf).astype(BF16)
    xpack = np.ascontiguousarray(xpack)

    # per-node (count>1)/max(count,eps), laid out [slot, block] per core
    rz_node = np.where(counts > 1, 1.0 / np.maximum(counts, EPS), 0.0)
    rz_node = rz_node.astype(np.float32)
    rz_all = np.zeros((NBLK, P), np.float32)
    rz_all[blk, slot] = rz_node
    rz_all = rz_all.reshape(NCORES, NB, P)

    i16c = GB * cap // 16

    in_maps = []
    for c in range(NCORES):
        tb = tgtq[c * NB:(c + 1) * NB]          # [NB, 2, cap]
        gi = gidxq[c * NB:(c + 1) * NB]
        # tgt columns: (group, half, block, tile) -> [P, C]
        tcore = (tb.reshape(ng, GB, NHALF, cap)
                 .transpose(0, 2, 1, 3)          # [ng, 2, GB, cap]
                 .reshape(ng * NHALF * GB * th, P).T)
        # idx16: per (group, half): stream of GB*cap idxs wrapped %16
        gs = (gi.reshape(ng, GB, NHALF, cap)
              .transpose(0, 2, 1, 3)             # [ng, 2, GB, cap]
              .reshape(ng * NHALF, GB * cap))    # per-gather streams
        idx16 = np.ascontiguousarray(
            np.tile(gs.reshape(ng * NHALF, i16c, 16).transpose(0, 2, 1)
                    .reshape(ng * NHALF * 16, i16c)
                    .reshape(ng * NHALF, 16, i16c)
                    .transpose(1, 0, 2).reshape(16, ng * NHALF * i16c),
                    (8, 1)))
        in_maps.append({
            "xpack": xpack,
            "gidx": idx16,
            "tgt": np.ascontiguousarray(tcore),
            "rz": np.ascontiguousarray(rz_all[c].T),   # [P, NB]
        })
    return th, in_maps, blk, slot


def _run(x, edge_index, trace=False):
    from concourse.bass_utils import run_bass_kernel_spmd

    th, in_maps, blk, slot = _host_prep(x, edge_index)
    key = ("prog", th)
    if key not in _CACHE:
        nc_ = _build_program(N_FEAT, NB, th, GB, NH)
        nc_.finalize()
        _CACHE[key] = nc_
    nc = _CACHE[key]
    res = run_bass_kernel_spmd(
        nc, in_maps, core_ids=list(range(NCORES)), trace=trace)

    # out layout: [ng*P, GB*f]; block b = g*GB + bl lives at rows g*P + slot,
    # cols bl*f:(bl+1)*f
    out_full = np.empty((N_NODES, N_FEAT), np.float32)
    ng = NB // GB
    cores = blk // NB
    for c in range(NCORES):
        o = np.asarray(res.results[c]["out"]).reshape(ng, P, GB, N_FEAT)
        m = cores == c
        bc = blk[m] % NB
        out_full[m] = o[bc // GB, slot[m], bc % GB]
    return out_full, res


def kernel(**inputs):
    out, _ = _run(inputs["x"], inputs["edge_index"], trace=False)
    return out


# revision 8
# speedup vs baseline: 1.3819x; 1.3819x over previous
"""GNN message-passing (std aggregator) on 8 TRN2 NeuronCores.

Math per target node: count, S1 = sum x[src], S2 = sum x[src]^2;
mean = S1/max(count,eps); var = S2/count - mean^2;
std = sqrt(max(var,0)), zeroed where count <= 1.

Strategy: shard TARGET nodes across cores (no collectives). Host packs nodes
into 128-bin blocks balanced by in-degree (serpentine deal), buckets edges by
(block, src-half) with uniform tile capacity th per (block,half) so one NEFF
serves all cores. Source rows are addressed with the int16 OFFSET trick:
gather base = table + (32768 + h*50000) rows, idx in [-32768, 17231], so each
half covers 50000 rows (vs 25000 with unsigned int16), cutting slot padding
from 25% to 13%. Host pre-packs a [N,128] bf16 table xpack = [x | x^2]
(256B gather rows) and per-node rz = (count>1)/max(count,eps).
Device per core, per group of GB blocks:
  - 2x dma_gather on alternating swdge queue pairs ({0,1} / {2,3} by group
    parity) so all 4 Q7 core pairs generate descriptors concurrently,
  - DVE builds 4-wide one-hot tiles (slot-vs-iota is_equal),
  - PE matmul-accumulates [128 bins x 128] = [S1 | S2] in PSUM,
  - ACT applies rz while copying PSUM->SBUF, DVE batched var, ACT sqrt,
    one DMA out per group.
"""

import numpy as np
import ml_dtypes

N_NODES = 100000
N_FEAT = 64
N_EDGES = 1600000
P = 128
NCORES = 8
NB = 98                 # blocks per core
NBLK = NCORES * NB      # 784
GB = 7                  # blocks per group; 98 = 14*7
NHALF = 2
NH = N_NODES // NHALF   # 50000 rows per half (int16 offset trick)
IOFF = 32768            # gather base offset rows
TROWS = IOFF + NH + 32768  # padded table rows: max addr = 32768+50000+32767
EPS = 1e-8
BF16 = ml_dtypes.bfloat16

_CACHE = {}


def _build_program(f, nb, th, gb, nh):
    import concourse.bass as bass
    import concourse.bacc as bacc
    import concourse.mybir as mybir
    import concourse.tile as tile

    F32 = mybir.dt.float32
    I16 = mybir.dt.int16
    BF = mybir.dt.bfloat16
    AO = mybir.AluOpType

    w = 2 * f                  # 128 = [x | x^2]
    t = NHALF * th             # tiles (columns of 128 edges) per block
    C = nb * t                 # total 128-edge packets per core
    gcols = gb * t             # packets per group
    qcols = gb * th            # packets per (group, half)
    ng = nb // gb
    nidx = qcols * P           # indices per gather
    i16c = nidx // 16          # idx16 cols per gather

    nc = bacc.Bacc(num_swdge_queues=4)
    xd = nc.declare_dram_parameter("xpack", [TROWS, w], BF, isOutput=False)
    gidxd = nc.declare_dram_parameter(
        "gidx", [P, ng * NHALF * i16c], I16, isOutput=False)
    tgtd = nc.declare_dram_parameter("tgt", [P, C], F32, isOutput=False)
    rzd = nc.declare_dram_parameter("rz", [P, nb], F32, isOutput=False)
    outd = nc.declare_dram_parameter("out", [ng * P, gb * f], F32,
                                     isOutput=True)

    with tile.TileContext(nc) as tc:
        with (
            tc.tile_pool(name="const", bufs=1) as constp,
            tc.tile_pool(name="io", bufs=4) as iop,
            tc.tile_pool(name="msg", bufs=4) as msgp,
            tc.tile_pool(name="oh", bufs=12) as ohp,
            tc.tile_pool(name="fin", bufs=2) as finp,
            tc.tile_pool(name="ov", bufs=2) as ovp,
            tc.tile_pool(name="ps", bufs=8, space="PSUM") as psump,
        ):
            # 4-wide iota [128, 4*128]: value = column index % 128
            iota4 = constp.tile([P, 4 * P], F32)
            nc.gpsimd.iota(iota4[:], pattern=[[0, 4], [1, P]], base=0,
                           channel_multiplier=0,
                           allow_small_or_imprecise_dtypes=True)
            rz = constp.tile([P, nb], F32)
            nc.sync.dma_start(out=rz[:], in_=rzd[:, :])

            for g in range(ng):
                idx = iop.tile([P, NHALF * i16c], I16, tag="idx")
                tg = iop.tile([P, gcols], F32, tag="tg")
                nc.sync.dma_start(
                    out=idx[:],
                    in_=gidxd[:, g * NHALF * i16c:(g + 1) * NHALF * i16c])
                nc.sync.dma_start(
                    out=tg[:], in_=tgtd[:, g * gcols:(g + 1) * gcols])

                sqx = msgp.tile([P, gcols * w], BF, tag="sqx")
                s3 = sqx[:].rearrange("p (c e) -> p c e", e=w)
                for h in range(NHALF):
                    base = IOFF + h * nh
                    nc.gpsimd.dma_gather(
                        out_ap=s3[:, h * qcols:(h + 1) * qcols, :],
                        in_ap=xd[base:base + 2, :],
                        idxs_ap=idx[:, h * i16c:(h + 1) * i16c],
                        num_idxs=nidx,
                        num_idxs_reg=nidx,
                        elem_size=w,
                        single_packet=False,
                        queue_num=2 * (g % 2) + h,
                    )

                pss = [psump.tile([P, w], F32, tag="ps", name=f"ps_{g}_{bl}")
                       for bl in range(gb)]
                for pk in range((gcols + 3) // 4):
                    npk = min(4, gcols - 4 * pk)
                    oh4 = ohp.tile([P, 4 * P], BF)
                    nc.vector.tensor_tensor(
                        out=oh4[:, 0:npk * P]
                            .rearrange("p (c e) -> p c e", e=P),
                        in0=tg[:, 4 * pk:4 * pk + npk]
                            .rearrange("p (c u) -> p c u", u=1)
                            .to_broadcast([P, npk, P]),
                        in1=iota4[:, 0:npk * P]
                            .rearrange("p (c e) -> p c e", e=P),
                        op=AO.is_equal,
                    )
                    for i in range(npk):
                        cl = 4 * pk + i
                        h = cl // qcols
                        r = cl % qcols
                        bl = r // th
                        j = r % th
                        nc.tensor.matmul(
                            out=pss[bl][:],
                            lhsT=oh4[:, i * P:(i + 1) * P],
                            rhs=sqx[:, cl * w:(cl + 1) * w],
                            start=(h == 0 and j == 0),
                            stop=(h == NHALF - 1 and j == th - 1),
                        )

                # finishing: ACT copies PSUM->SBUF scaled by rz, then DVE
                # batched var over [P, gb*f], ACT sqrt, one DMA per group
                me = finp.tile([P, gb * w], F32, tag="me")
                m3 = me[:].rearrange("p (b e) -> p b e", e=w)
                for bl in range(gb):
                    b = g * gb + bl
                    nc.scalar.mul(
                        out=me[:, bl * w:(bl + 1) * w], in_=pss[bl][:],
                        mul=rz[:, b:b + 1])
                var = finp.tile([P, gb * f], F32, tag="var")
                v3 = var[:].rearrange("p (b e) -> p b e", e=f)
                nc.vector.tensor_tensor(
                    out=v3[:, :, :], in0=m3[:, :, 0:f], in1=m3[:, :, 0:f],
                    op=AO.mult)
                nc.vector.tensor_tensor(
                    out=v3[:, :, :], in0=m3[:, :, f:w], in1=v3[:, :, :],
                    op=AO.subtract)
                nc.vector.tensor_scalar(
                    out=var[:], in0=var[:], scalar1=0.0, scalar2=None,
                    op0=AO.max)
                std = ovp.tile([P, gb * f], F32, tag="std")
                nc.scalar.sqrt(out=std[:], in_=var[:])
                nc.sync.dma_start(
                    out=outd[g * P:(g + 1) * P, :], in_=std[:])
    return nc


def _host_prep(x, edge_index):
    src = np.asarray(edge_index[0], dtype=np.int64)
    tgt = np.asarray(edge_index[1], dtype=np.int64)
    n_edges = src.shape[0]
    counts = np.bincount(tgt, minlength=N_NODES)

    # serpentine deal of count-sorted nodes into NBLK blocks of <=128 slots
    order = np.argsort(-counts, kind="stable")
    ranks = np.arange(N_NODES)
    rounds = ranks // NBLK
    pos = ranks % NBLK
    blk_of_rank = np.where(rounds % 2 == 0, pos, NBLK - 1 - pos)
    blk = np.empty(N_NODES, np.int64)
    slot = np.empty(N_NODES, np.int64)
    blk[order] = blk_of_rank
    slot[order] = rounds
    assert slot.max() < P

    eb = blk[tgt]                      # edge -> block
    eh = src // NH                     # edge -> src half
    es = slot[tgt]                     # edge -> slot in block
    seg = eb * NHALF + eh              # edge -> (block, half) segment
    segsums = np.bincount(seg, minlength=NBLK * NHALF)
    th = int(np.ceil(segsums.max() / P))
    cap = th * P

    # within each segment, order edges by src row for DRAM gather locality
    order_e = np.lexsort((src, seg))
    segs = seg[order_e]
    starts = np.zeros(NBLK * NHALF, np.int64)
    np.cumsum(segsums[:-1], out=starts[1:])
    within = np.arange(n_edges) - starts[segs]
    flat = segs * cap + within

    # idx values use the int16 offset trick: row r of half -> r - 32768;
    # padding slots use 0 (a valid row; one-hot column is all-zero)
    gidxq = np.zeros((NBLK, NHALF, cap), np.int16)
    tgtq = np.full((NBLK, NHALF, cap), -1.0, np.float32)
    gidxq.reshape(-1)[flat] = (src[order_e] % NH - IOFF).astype(np.int16)
    tgtq.reshape(-1)[flat] = es[order_e].astype(np.float32)

    # trailing-pop guard: the gather ucode drops trailing negative idxs from
    # each stream; ensure the final slot of every (core, group, half) stream
    # (= last block of the group, tile th-1, pos 127) has idx >= 0 by
    # swapping within its segment (edges may occupy any slot of their seg).
    ng = NB // GB
    for c in range(NCORES):
        for g in range(ng):
            b = c * NB + g * GB + GB - 1
            for h in range(NHALF):
                if gidxq[b, h, cap - 1] < 0 and tgtq[b, h, cap - 1] >= 0:
                    cand = np.nonzero(gidxq[b, h] >= 0)[0]
                    assert cand.size > 0, "no swap partner for pop guard"
                    jj = cand[0]
                    gidxq[b, h, cap - 1], gidxq[b, h, jj] = (
                        gidxq[b, h, jj], gidxq[b, h, cap - 1])
                    tgtq[b, h, cap - 1], tgtq[b, h, jj] = (
                        tgtq[b, h, jj], tgtq[b, h, cap - 1])

    # packed per-node table [x | x^2] in bf16 (256B rows), padded for the
    # offset addressing window
    xf = np.asarray(x, dtype=np.float32)
    xpack = np.zeros((TROWS, 2 * N_FEAT), BF16)
    xpack[:N_NODES, :N_FEAT] = xf.astype(BF16)
    xpack[:N_NODES, N_FEAT:] = (xf * xf).astype(BF16)
    xpack = np.ascontiguousarray(xpack)

    # per-node (count>1)/max(count,eps), laid out [slot, block] per core
    rz_node = np.where(counts > 1, 1.0 / np.maximum(counts, EPS), 0.0)
    rz_node = rz_node.astype(np.float32)
    rz_all = np.zeros((NBLK, P), np.float32)
    rz_all[blk, slot] = rz_node
    rz_all = rz_all.reshape(NCORES, NB, P)

    i16c = GB * cap // 16

    in_maps = []
    for c in range(NCORES):
        tb = tgtq[c * NB:(c + 1) * NB]          # [NB, 2, cap]
        gi = gidxq[c * NB:(c + 1) * NB]
        # tgt columns: (group, half, block, tile) -> [P, C]
        tcore = (tb.reshape(ng, GB, NHALF, cap)
                 .transpose(0, 2, 1, 3)          # [ng, 2, GB, cap]
                 .reshape(ng * NHALF * GB * th, P).T)
        # idx16: per (group, half): stream of GB*cap idxs wrapped %16
        gs = (gi.reshape(ng, GB, NHALF, cap)
              .transpose(0, 2, 1, 3)             # [ng, 2, GB, cap]
              .reshape(ng * NHALF, GB * cap))    # per-gather streams
        idx16 = np.ascontiguousarray(
            np.tile(gs.reshape(ng * NHALF, i16c, 16).transpose(0, 2, 1)
                    .reshape(ng * NHALF * 16, i16c)
                    .reshape(ng * NHALF, 16, i16c)
                    .transpose(1, 0, 2).reshape(16, ng * NHALF * i16c),
                    (8, 1)))
        in_maps.append({
            "xpack": xpack,
            "gidx": idx16,
            "tgt": np.ascontiguousarray(tcore),
            "rz": np.ascontiguousarray(rz_all[c].T),   # [P, NB]
        })
    return th, in_maps, blk, slot


def _run(x, edge_index, trace=False):
    from concourse.bass_utils import run_bass_kernel_spmd

    th, in_maps, blk, slot = _host_prep(x, edge_index)
    key = ("prog", th)
    if key not in _CACHE:
        nc_ = _build_program(N_FEAT, NB, th, GB, NH)
        nc_.finalize()
        _CACHE[key] = nc_
    nc = _CACHE[key]
    res = run_bass_kernel_spmd(
        nc, in_maps, core_ids=list(range(NCORES)), trace=trace)

    # out layout: [ng*P, GB*f]; block b = g*GB + bl lives at rows g*P + slot,
    # cols bl*f:(bl+1)*f
    out_full = np.empty((N_NODES, N_FEAT), np.float32)
    ng = NB // GB
    cores = blk // NB
    for c in range(NCORES):
        o = np.asarray(res.results[c]["out"]).reshape(ng, P, GB, N_FEAT)
        m = cores == c
        bc = blk[m] % NB
        out_full[m] = o[bc // GB, slot[m], bc % GB]
    return out_full, res


def kernel(**inputs):
    out, _ = _run(inputs["x"], inputs["edge_index"], trace=False)
    return out
